# revision 16
# baseline (speedup 1.0000x reference)
"""Trainium2 Bass kernel for nn_MultiHeadTokenAttention — v3.

v3 (default when every (b,s) has <= 96 unmasked tokens; else falls back to
the dense v2 path below): mask-sparsity packing.  The binary mask kills
~half the t positions (softmax weight exp(-1e4) ~ 0), so the host packs
only the unmasked rows of each segment (b, s), padded to a uniform TP=96,
4 segments per 3 x 128-partition blocks (s0..s2 at rows [0:96] of blocks
0..2, s3 split into three 32-row strips at rows [96:128]).  scores and the
V projection then run on 48 instead of 64 row-blocks (-25% PE columns).
Per-segment softmax sums use 0/1 segment-indicator matmuls (z97 variant
keeps slots at partitions {0,32,64,96}); the 1/Z broadcast back to rows is
a [4,512]-stationary matmul.  attn.V runs per segment as partition-range
matmuls: K=96 at base 0 for s0..s2, and 3 x K=32 at base 96 for s3 (the
96 base requires an explicit tile_position=(96, 0) — the auto path only
allows bases {0,32,64}).  Padding rows enter exp with a -1e4 bias so their
alpha is exactly 0.  Everything else (rt gather, O projection, LayerNorm,
s-major store) is unchanged from v2.

Measured on the 8-core axon fleet: ~0.51-0.56 ms/iter steady-state vs
~0.59-0.62 ms for tuned v2 (and 1.26-1.87 ms for the prior session's
wall-clock-based measurement of the same v2 kernel).  Rel err 5.3e-3.

--- v2 notes (dense path, kept as fallback) ---

Reference computation (per batch element b):
    q = ini_q @ Wq.T + bq                      [Q, H] -> heads [Q, 16, 64]
    k = X @ Wk.T + bk ;  v = X @ Wv.T + bv     (X = ini_k[b] as [S*T, H])
    scores[h,q,s,t] = (q_h . k_h) / 8,  + mask*-1e4, softmax over t
    res[q,s,:] = concat_h(sum_t alpha * v_h)   [Q, S, H]
    res = res @ Wo.T + bo;  LayerNorm(res) * gamma + beta

Sharding: batch-parallel, one batch element per NeuronCore (8 cores, no
collectives).

v2 structure (host + device):
  * Host folds Wk into the queries:  qk[32h+q, :] = scale * q_h @ Wk_h
    so  scoresT[st, hq] = X @ qk^T  -- the K projection is never
    materialized (4.3G MACs instead of 8.9G) and no K^T is needed.
  * Host pre-transposes X to bf16 X^T, tiled [16 chunks][128 p][8 c][512]
    so each chunk's load is one fully-contiguous 1 MB DMA and the device
    does zero transposes (PE transposes were ~20% of baseline PE time).
  * Softmax runs in the transposed layout [t on partitions, hq free]:
    mask enters as the per-partition bias of the exp activation; column
    sums via a ones-stationary matmul; 1/Z broadcast across partitions
    via a K=1 matmul; one DVE multiply -> alphaT (bf16).
  * attn.V: lhsT = V_s [128 t, 128 hd (2 heads)], rhs = alphaT cols of
    the same 2 heads -> out [128 hd, 64]; diagonal 64x32 blocks are the
    valid res^T entries, gathered by 2 strided DVE copies per head-pair
    into rt_c [128 hd, 128 (s,q)] which feeds the O projection as its
    stationary operand unchanged.  LayerNorm as in v1.

All matmuls run bf16 (f32 PSUM accumulate); rel err vs f32 reference is
~2e-3, comfortably under the 2e-2 gate.
"""

import os
import sys

for _p in ("/opt/trn_rl_repo", "/root/.axon_site/_ro/trn_rl_repo"):
    if os.path.isdir(_p) and _p not in sys.path:
        sys.path.insert(0, _p)

import numpy as np

B, Q, S, T, H = 8, 32, 64, 128, 1024
HEADS, D = 16, 64
ST = S * T           # 8192 rows of X per batch element
NCORES = 8
NG = 16              # chunks per core (4 s-values = 512 st rows each)
HQ = HEADS * Q       # 512
EPS = 1e-12

_BUILD_CACHE = {}

# softmax 1/Z plumbing: "mm" = colsum+broadcast via PE matmuls;
# "ar" = gpsimd partition_all_reduce + DVE recip/mult (no PE work, no PSUM)
ZMODE = os.environ.get("KV2_ZMODE", "ar")
IOP_BUFS = int(os.environ.get("KV2_IOP_BUFS", "3"))
V3_BCAST = os.environ.get("KV3_BCAST", "mm")  # "mm" matmul | "ps" pool-bcast
V3_PP = os.environ.get("KV3_PP", "")  # "mm,zzb,av" PSUM ring sizes
V3_GATHER = os.environ.get("KV3_GATHER", "dve")  # dve | pool | split
V3_ZLATE = int(os.environ.get("KV3_ZLATE", "0"))  # 1: z/zb after V block 0
V3_OINT = int(os.environ.get("KV3_OINT", "0"))   # 1: interleave O with attnV
V3_S3PACK = int(os.environ.get("KV3_S3PACK", "1"))  # 1: repack s3 strips (506us vs 527us)
SMP_BUFS = int(os.environ.get("KV2_SMP_BUFS", "2"))
LNP_BUFS = int(os.environ.get("KV2_LNP_BUFS", "2"))
MM_BUFS = int(os.environ.get("KV2_MM_BUFS", "0"))  # 0 -> default
if not MM_BUFS and ZMODE == "ar":
    MM_BUFS = 6  # measured: 590us vs 614us at 5


def _build(bias_kq=False, bias_v=False, bias_o=False, gamma_beta=False,
           loop=1, stages=9):
    """Build + compile the Bass program. Returns the Bacc object."""
    import concourse.mybir as mybir
    from concourse import bacc
    from concourse.tile import TileContext

    f32 = mybir.dt.float32
    bf16 = mybir.dt.bfloat16
    ADD = mybir.AluOpType.add
    SUB = mybir.AluOpType.subtract
    MULT = mybir.AluOpType.mult
    AXX = mybir.AxisListType.X
    EXP = mybir.ActivationFunctionType.Exp
    LN_F = mybir.ActivationFunctionType.Ln
    DIV = mybir.AluOpType.divide
    from concourse import bass_isa

    nc = bacc.Bacc("TRN2", target_bir_lowering=False, debug=False,
                   num_devices=NCORES)

    # X^T bf16, tiled: xkt[g, p, c, j] = X[512 g + j, 128 c + p]
    xkt_d = nc.dram_tensor("xkt", [NG, 128, 8, 512], bf16,
                           kind="ExternalInput")
    # qk^T bf16: qkt[c, p, m] = qk[m, 128 c + p]  (m = 32 h + q)
    qkt_d = nc.dram_tensor("qkt", [8, 128, HQ], bf16, kind="ExternalInput")
    # mask^T * -1e4: mnegt[t, s]
    mnegt_d = nc.dram_tensor("mnegt", [T, S], f32, kind="ExternalInput")
    # Wv^T bf16 rows h cols hd; Wo^T bf16 rows hd cols H
    wvt_d = nc.dram_tensor("wvt", [H, H], bf16, kind="ExternalInput")
    wot_d = nc.dram_tensor("wot", [H, H], bf16, kind="ExternalInput")
    bkq_d = nc.dram_tensor("bkq", [1, HQ], bf16, kind="ExternalInput")
    bv_d = nc.dram_tensor("bvr", [1, H], bf16, kind="ExternalInput")
    bo_d = nc.dram_tensor("bor", [1, H], bf16, kind="ExternalInput")
    gam_d = nc.dram_tensor("gam", [1, H], f32, kind="ExternalInput")
    bet_d = nc.dram_tensor("bet", [1, H], f32, kind="ExternalInput")
    # s-major output: contiguous 512 KB write per chunk (the q-major layout
    # costs 128 scattered 4 KB descriptors per chunk and dominates the
    # critical path); host returns a transposed view.
    out_d = nc.dram_tensor("out", [S, Q, H], f32, kind="ExternalOutput")

    mm_bufs = MM_BUFS or (3 if ZMODE == "mm" else 5)
    with TileContext(nc) as tc:
        with tc.tile_pool(name="wts", bufs=1) as wpool, \
             tc.tile_pool(name="ppmm", bufs=mm_bufs, space="PSUM") as ppmm, \
             tc.tile_pool(name="ppz", bufs=1, space="PSUM") as ppz, \
             tc.tile_pool(name="ppzb", bufs=2, space="PSUM") as ppzb, \
             tc.tile_pool(name="ppav", bufs=2, space="PSUM") as ppav:

            # ---------------- preamble: constants + weights ----------------
            eps_sb = wpool.tile([128, 1], f32, name="eps_sb")
            nc.vector.memset(eps_sb[:], EPS)
            ones_col = wpool.tile([128, 1], bf16, name="ones_col")
            nc.vector.memset(ones_col[:], 1.0)
            ones_row = wpool.tile([1, 128], bf16, name="ones_row")
            nc.vector.memset(ones_row[:], 1.0)
            ones_row_f = wpool.tile([1, 128], f32, name="ones_row_f")
            nc.vector.memset(ones_row_f[:], 1.0)

            mneg_sb = wpool.tile([T, S], f32, name="mneg_sb")
            nc.sync.dma_start(mneg_sb[:], mnegt_d[:])

            qk_sb, wv_sb, wo_sb = [], [], []
            for c in range(8):
                qkc = wpool.tile([128, HQ], bf16, name=f"qk{c}")
                nc.gpsimd.dma_start(qkc[:], qkt_d[c])
                qk_sb.append(qkc)
                wvc = wpool.tile([128, H], bf16, name=f"wv{c}")
                nc.gpsimd.dma_start(wvc[:], wvt_d[128 * c:128 * (c + 1), :])
                wv_sb.append(wvc)
                woc = wpool.tile([128, H], bf16, name=f"wo{c}")
                nc.gpsimd.dma_start(woc[:], wot_d[128 * c:128 * (c + 1), :])
                wo_sb.append(woc)

            if bias_kq:
                bkq_sb = wpool.tile([1, HQ], bf16, name="bkq_sb")
                nc.gpsimd.dma_start(bkq_sb[:], bkq_d[:])
            if bias_v:
                bv_sb = wpool.tile([1, H], bf16, name="bv_sb")
                nc.gpsimd.dma_start(bv_sb[:], bv_d[:])
            if bias_o:
                bo_sb = wpool.tile([1, H], bf16, name="bo_sb")
                nc.gpsimd.dma_start(bo_sb[:], bo_d[:])
            if gamma_beta:
                gam_sb = wpool.tile([128, H], f32, name="gam_sb")
                bet_sb = wpool.tile([128, H], f32, name="bet_sb")
                nc.sync.dma_start(
                    gam_sb[:], gam_d[0, :].partition_broadcast(128))
                nc.sync.dma_start(
                    bet_sb[:], bet_d[0, :].partition_broadcast(128))

            # ---------------- main per-chunk pipeline ----------------
            with tc.tile_pool(name="io", bufs=IOP_BUFS) as iop, \
                 tc.tile_pool(name="sm", bufs=SMP_BUFS) as smp, \
                 tc.tile_pool(name="ln", bufs=LNP_BUFS) as lnp:

                def emit_chunk(g):
                    # 1. load X^T chunk: one contiguous 1 MB DMA.  Issued on
                    # the (otherwise idle) Pool queue so it never queues
                    # behind the out-store on SP.
                    xt = iop.tile([128, 4096], bf16, name="xt")
                    nc.sync.dma_start(
                        xt[:].rearrange("p (c j) -> p c j", c=8), xkt_d[g])

                    ex_t, al_t, v_t = [], [], []
                    for sp in range(4):
                        # 2. scoresT[t, hq] for s = 4g+sp
                        ps = ppmm.tile([128, 512], f32, name="ps", tag="mm")
                        for c in range(8):
                            nc.tensor.matmul(
                                ps[:], xt[:, 512 * c + 128 * sp:
                                          512 * c + 128 * (sp + 1)],
                                qk_sb[c][:],
                                start=(c == 0),
                                stop=(c == 7 and not bias_kq))
                        if bias_kq:
                            nc.tensor.matmul(ps[:], ones_row[:], bkq_sb[:],
                                             start=False, stop=True)
                        # 3. exp(scoresT + mask_col) -> bf16, mask via bias
                        ex = smp.tile([128, 512], bf16, name=f"ex{sp}")
                        nc.scalar.activation(
                            ex[:], ps[:], EXP,
                            bias=mneg_sb[:, 4 * g + sp:4 * g + sp + 1])
                        ex_t.append(ex)
                        al = smp.tile([128, 512], bf16, name=f"al{sp}")
                        if ZMODE == "ar":
                            # 4+5. Z bcast via gpsimd all-reduce;
                            # al = ex * (1/Z)  (DVE divide is not valid ISA)
                            zsb = smp.tile([128, 512], f32, name="zsb",
                                           tag="zsb", bufs=2)
                            nc.gpsimd.partition_all_reduce(
                                zsb[:], ex[:], 128, bass_isa.ReduceOp.add)
                            zrb = smp.tile([128, 512], f32, name="zrb",
                                           tag="zrb", bufs=2)
                            nc.vector.reciprocal(zrb[:], zsb[:])
                            nc.vector.tensor_tensor(al[:], ex[:], zrb[:],
                                                    MULT)
                        else:
                            # 4. Z[hq] colsums via ones-stationary matmul
                            z = ppz.tile([1, 512], f32, name="z", tag="z")
                            nc.tensor.matmul(z[:], ones_col[:], ex[:],
                                             start=True, stop=True)
                            zr = smp.tile([1, 512], f32, name="zr",
                                          tag="zr", bufs=2)
                            nc.vector.reciprocal(zr[:], z[:])
                            # 5. bcast 1/Z across partitions via K=1 matmul
                            zb = ppzb.tile([128, 512], f32, name="zb",
                                           tag="zb")
                            nc.tensor.matmul(zb[:], ones_row_f[:], zr[:],
                                             start=True, stop=True)
                            nc.vector.tensor_tensor(al[:], ex[:], zb[:],
                                                    MULT)
                        al_t.append(al)
                        # 6. V_s[t, hd] natural
                        vs = smp.tile([128, H], bf16, name=f"v{sp}")
                        for n in range(2):
                            pv = ppmm.tile([128, 512], f32, name="pv",
                                           tag="mm")
                            for c in range(8):
                                nc.tensor.matmul(
                                    pv[:],
                                    xt[:, 512 * c + 128 * sp:
                                       512 * c + 128 * (sp + 1)],
                                    wv_sb[c][:, 512 * n:512 * (n + 1)],
                                    start=(c == 0),
                                    stop=(c == 7 and not bias_v))
                            if bias_v:
                                nc.tensor.matmul(
                                    pv[:], ones_row[:],
                                    bv_sb[:, 512 * n:512 * (n + 1)],
                                    start=False, stop=True)
                            nc.scalar.copy(vs[:, 512 * n:512 * (n + 1)],
                                           pv[:])
                        v_t.append(vs)

                    if stages < 4:
                        osb_stub = lnp.tile([128, H], f32, name="osb")
                        nc.vector.tensor_copy(osb_stub[:, 0:512], al_t[0][:])
                        nc.vector.tensor_copy(osb_stub[:, 512:1024],
                                              v_t[3][:, 0:512])
                        nc.sync.dma_start(out_d[4 * g:4 * (g + 1)],
                                          osb_stub[:])
                        return

                    # 7. attn.V -> rt_c [128 hd (2 heads), 128 (s, q)]
                    # pav is a full PSUM bank: a half-bank tile would share
                    # its physical bank with the pool's other rotation buf,
                    # and PE-write + DVE-read of one bank is a fatal HW
                    # PSUM collision (not modeled by CoreSim).
                    rt_t = []
                    for c in range(8):
                        pav = ppav.tile([128, 512], f32, name="pav",
                                        tag="av")
                        for sp in range(4):
                            nc.tensor.matmul(
                                pav[:, 64 * sp:64 * (sp + 1)],
                                v_t[sp][:, 128 * c:128 * (c + 1)],
                                al_t[sp][:, 64 * c:64 * (c + 1)],
                                start=True, stop=True)
                        rt = smp.tile([128, 128], bf16, name=f"rt{c}")
                        eng_a = (nc.vector if V3_GATHER in ("dve", "split")
                                 else nc.gpsimd)
                        eng_b = (nc.vector if V3_GATHER == "dve"
                                 else nc.gpsimd)
                        eng_a.tensor_copy(
                            rt[0:64, :].rearrange("p (s q) -> p s q", q=32),
                            pav[0:64, 0:256]
                            .rearrange("p (s q2) -> p s q2", q2=64)[:, :, 0:32])
                        eng_b.tensor_copy(
                            rt[64:128, :].rearrange("p (s q) -> p s q", q=32),
                            pav[64:128, 0:256]
                            .rearrange("p (s q2) -> p s q2", q2=64)[:, :, 32:64])
                        rt_t.append(rt)

                    if stages < 6:
                        osb_stub = lnp.tile([128, H], f32, name="osb")
                        nc.vector.tensor_copy(osb_stub[:, 0:64],
                                              rt_t[0][:].bitcast(f32))
                        nc.vector.tensor_copy(osb_stub[:, 64:128],
                                              rt_t[7][:].bitcast(f32))
                        nc.vector.tensor_copy(osb_stub[:, 128:256],
                                              osb_stub[:, 0:128])
                        nc.vector.tensor_copy(osb_stub[:, 256:512],
                                              osb_stub[:, 0:256])
                        nc.vector.tensor_copy(osb_stub[:, 512:1024],
                                              osb_stub[:, 0:512])
                        nc.sync.dma_start(out_d[4 * g:4 * (g + 1)],
                                          osb_stub[:])
                        return

                    # 8. O-proj: rows (s, q) on partitions, H on free
                    osb = lnp.tile([128, H], f32, name="osb")
                    for n in range(2):
                        po = ppmm.tile([128, 512], f32, name="po", tag="mm")
                        for c in range(8):
                            nc.tensor.matmul(
                                po[:], rt_t[c][:],
                                wo_sb[c][:, 512 * n:512 * (n + 1)],
                                start=(c == 0),
                                stop=(c == 7 and not bias_o))
                        if bias_o:
                            nc.tensor.matmul(
                                po[:], ones_row[:],
                                bo_sb[:, 512 * n:512 * (n + 1)],
                                start=False, stop=True)
                        nc.scalar.copy(osb[:, 512 * n:512 * (n + 1)], po[:])

                    # 9. LayerNorm over H (in place on osb).  rstd via the
                    # DVE pow ALU op, so ACT only ever needs exp+copy (one
                    # act-table load, hoisted out of the loop).
                    s1 = lnp.tile([128, 1], f32, name="s1")
                    nc.vector.tensor_reduce(s1[:], osb[:], axis=AXX, op=ADD)
                    mean = lnp.tile([128, 1], f32, name="mean")
                    nc.vector.tensor_scalar(mean[:], s1[:], 1.0 / H, None,
                                            MULT)
                    nc.vector.tensor_scalar(osb[:], osb[:], mean[:], None,
                                            SUB)
                    sq = lnp.tile([128, H], f32, name="sq")
                    nc.vector.tensor_tensor(sq[:], osb[:], osb[:], MULT)
                    ssq = lnp.tile([128, 1], f32, name="ssq")
                    nc.vector.tensor_reduce(ssq[:], sq[:], axis=AXX, op=ADD)
                    # ln(ssq/H + eps) via the activation's scale+bias, then
                    # rstd = exp(-0.5 ln(var+eps)); both funcs live in the
                    # pinned act table set.
                    lnv = lnp.tile([128, 1], f32, name="lnv")
                    nc.scalar.activation(lnv[:], ssq[:], LN_F,
                                         bias=eps_sb[:], scale=1.0 / H)
                    rstd = lnp.tile([128, 1], f32, name="rstd")
                    nc.scalar.activation(rstd[:], lnv[:], EXP, scale=-0.5)
                    nc.vector.tensor_scalar(osb[:], osb[:], rstd[:], None,
                                            MULT)
                    if gamma_beta:
                        nc.vector.tensor_tensor(osb[:], osb[:], gam_sb[:],
                                                MULT)
                        nc.vector.tensor_tensor(osb[:], osb[:], bet_sb[:],
                                                ADD)

                    # 10. out[4g:4g+4, :, :] <- rows (s-major, q); fully
                    # contiguous 512 KB store.  Issued via SWDGE (Pool
                    # queue) so the next chunk's xt load on SP never queues
                    # behind it.
                    nc.gpsimd.dma_start(out_d[4 * g:4 * (g + 1)], osb[:])

                def emit_all():
                    for g in range(NG):
                        emit_chunk(g)

                if loop > 1:
                    with tc.For_i(0, loop, 1):
                        emit_all()
                else:
                    emit_all()

    # Pin exp/ln/copy activations to the one act-table set that holds all
    # of them ("natural_log_exp_and_others") so the table load is emitted
    # once and hoisted out of the loop instead of swapping every chunk.
    import concourse.bacc as bacc_mod
    _orig_gat = bacc_mod.get_activation_tables
    _pin = {mybir.ActivationFunctionType.Exp, mybir.ActivationFunctionType.Ln,
            mybir.ActivationFunctionType.Copy,
            mybir.ActivationFunctionType.Identity}

    def _gat(arch):
        tables = _orig_gat(arch)
        return {name: (funcs if name == "natural_log_exp_and_others"
                       else funcs - _pin)
                for name, funcs in tables.items()}

    bacc_mod.get_activation_tables = _gat
    try:
        nc.compile()
    finally:
        bacc_mod.get_activation_tables = _orig_gat
    return nc




# ---------------------------------------------------------------------------
# v3: mask-sparsity packing.  Each segment (b, s) keeps only its unmasked
# t-rows (plus zero padding) at uniform length TP=96 >= max unmasked count.
# Groups of 4 segments pack into 3 x 128-row blocks: s0,s1,s2 occupy rows
# [0:96] of blocks 0,1,2; s3 is split into three 32-row strips at rows
# [96:128] (PE matmul reaches base partition 96 via explicit tile_position).
# scores and the V projection then run on 48 instead of 64 row-blocks.
# Softmax Z per segment via segment-indicator matmuls:
#   z[4, 512]  = sum_b seg_b^T @ ex_b      (seg_b [128,4] 0/1 host constant)
#   zb_b[128, 512] = segT_b @ recip(z)     (broadcast back to rows)
# ---------------------------------------------------------------------------
TP = 96          # packed tokens per segment
NBLK = 48        # 16 groups x 3 blocks


def _build3(bias_kq=False, bias_v=False, bias_o=False, gamma_beta=False,
            loop=1, stages=9):
    import concourse.mybir as mybir
    from concourse import bacc
    from concourse.tile import TileContext

    f32 = mybir.dt.float32
    bf16 = mybir.dt.bfloat16
    ADD = mybir.AluOpType.add
    SUB = mybir.AluOpType.subtract
    MULT = mybir.AluOpType.mult
    AXX = mybir.AxisListType.X
    EXP = mybir.ActivationFunctionType.Exp
    LN_F = mybir.ActivationFunctionType.Ln

    nc = bacc.Bacc("TRN2", target_bir_lowering=False, debug=False,
                   num_devices=NCORES)

    # packed X^T bf16: xkt3[g, p, c, j] = Xp[g, j, 128c + p], j in [0, 384)
    xkt_d = nc.dram_tensor("xkt3", [16, 128, 8, 384], bf16,
                           kind="ExternalInput")
    qkt_d = nc.dram_tensor("qkt", [8, 128, HQ], bf16, kind="ExternalInput")
    # exp bias per (partition, block): 0 for real rows, -1e4 for padding
    mneg_d = nc.dram_tensor("mneg3", [128, NBLK], f32, kind="ExternalInput")
    # seg3 one-hot segment indicators; "ps" variant pads the segment slots
    # to partitions {0,32,64,96} of a [97, 512] PSUM tile so recip reads are
    # 32-aligned and gpsimd broadcast inputs sit at partition 0
    segw = 97 if V3_BCAST == "ps" else 4
    seg_d = nc.dram_tensor("seg3", [128, 3 * segw], bf16,
                           kind="ExternalInput")
    segt_d = nc.dram_tensor("segt3", [4, 384], bf16, kind="ExternalInput")
    wvt_d = nc.dram_tensor("wvt", [H, H], bf16, kind="ExternalInput")
    wot_d = nc.dram_tensor("wot", [H, H], bf16, kind="ExternalInput")
    bkq_d = nc.dram_tensor("bkq", [1, HQ], bf16, kind="ExternalInput")
    bv_d = nc.dram_tensor("bvr", [1, H], bf16, kind="ExternalInput")
    bo_d = nc.dram_tensor("bor", [1, H], bf16, kind="ExternalInput")
    gam_d = nc.dram_tensor("gam", [1, H], f32, kind="ExternalInput")
    bet_d = nc.dram_tensor("bet", [1, H], f32, kind="ExternalInput")
    out_d = nc.dram_tensor("out", [S, Q, H], f32, kind="ExternalOutput")

    if V3_PP:
        mm_bufs, z_bufs, av_bufs = (int(x) for x in V3_PP.split(","))
    else:
        mm_bufs = 5 if V3_BCAST == "ps" else 4
        z_bufs = 1 if V3_BCAST == "ps" else 2
        av_bufs = 2
    with TileContext(nc) as tc:
        with tc.tile_pool(name="wts", bufs=1) as wpool, \
             tc.tile_pool(name="ppmm", bufs=mm_bufs, space="PSUM") as ppmm, \
             tc.tile_pool(name="ppz", bufs=z_bufs, space="PSUM") as ppz, \
             tc.tile_pool(name="ppav", bufs=av_bufs, space="PSUM") as ppav:

            eps_sb = wpool.tile([128, 1], f32, name="eps_sb")
            nc.vector.memset(eps_sb[:], EPS)
            ones_row = wpool.tile([1, 128], bf16, name="ones_row")
            nc.vector.memset(ones_row[:], 1.0)

            mneg_sb = wpool.tile([128, NBLK], f32, name="mneg_sb")
            nc.sync.dma_start(mneg_sb[:], mneg_d[:])
            seg_sb = wpool.tile([128, 3 * segw], bf16, name="seg_sb")
            nc.sync.dma_start(seg_sb[:], seg_d[:])
            segt_sb = wpool.tile([4, 384], bf16, name="segt_sb")
            nc.sync.dma_start(segt_sb[:], segt_d[:])

            qk_sb, wv_sb, wo_sb = [], [], []
            for c in range(8):
                qkc = wpool.tile([128, HQ], bf16, name=f"qk{c}")
                nc.gpsimd.dma_start(qkc[:], qkt_d[c])
                qk_sb.append(qkc)
                wvc = wpool.tile([128, H], bf16, name=f"wv{c}")
                nc.gpsimd.dma_start(wvc[:], wvt_d[128 * c:128 * (c + 1), :])
                wv_sb.append(wvc)
                woc = wpool.tile([128, H], bf16, name=f"wo{c}")
                nc.gpsimd.dma_start(woc[:], wot_d[128 * c:128 * (c + 1), :])
                wo_sb.append(woc)

            if bias_kq:
                bkq_sb = wpool.tile([1, HQ], bf16, name="bkq_sb")
                nc.gpsimd.dma_start(bkq_sb[:], bkq_d[:])
            if bias_v:
                bv_sb = wpool.tile([1, H], bf16, name="bv_sb")
                nc.gpsimd.dma_start(bv_sb[:], bv_d[:])
            if bias_o:
                bo_sb = wpool.tile([1, H], bf16, name="bo_sb")
                nc.gpsimd.dma_start(bo_sb[:], bo_d[:])
            if gamma_beta:
                gam_sb = wpool.tile([128, H], f32, name="gam_sb")
                bet_sb = wpool.tile([128, H], f32, name="bet_sb")
                nc.sync.dma_start(
                    gam_sb[:], gam_d[0, :].partition_broadcast(128))
                nc.sync.dma_start(
                    bet_sb[:], bet_d[0, :].partition_broadcast(128))

            with tc.tile_pool(name="io", bufs=3) as iop, \
                 tc.tile_pool(name="sm", bufs=2) as smp, \
                 tc.tile_pool(name="ln", bufs=2) as lnp:

                def emit_chunk(g):
                    xt = iop.tile([128, 3072], bf16, name="xt")
                    nc.sync.dma_start(
                        xt[:].rearrange("p (c j) -> p c j", c=8), xkt_d[g])

                    # scores + exp per block
                    ex_t = []
                    for b in range(3):
                        ps = ppmm.tile([128, 512], f32, name="ps", tag="mm")
                        for c in range(8):
                            nc.tensor.matmul(
                                ps[:], xt[:, 384 * c + 128 * b:
                                          384 * c + 128 * (b + 1)],
                                qk_sb[c][:],
                                start=(c == 0),
                                stop=(c == 7 and not bias_kq))
                        if bias_kq:
                            nc.tensor.matmul(ps[:], ones_row[:], bkq_sb[:],
                                             start=False, stop=True)
                        ex = smp.tile([128, 512], bf16, name=f"ex{b}")
                        nc.scalar.activation(
                            ex[:], ps[:], EXP,
                            bias=mneg_sb[:, 3 * g + b:3 * g + b + 1])
                        ex_t.append(ex)

                    # Z per segment via indicator matmuls, then broadcast
                    al_t = []
                    v_t = []

                    def emit_vblock(b):
                        vs = smp.tile([128, H], bf16, name=f"v{b}")
                        for n in range(2):
                            pv = ppmm.tile([128, 512], f32, name="pv",
                                           tag="mm")
                            for c in range(8):
                                nc.tensor.matmul(
                                    pv[:],
                                    xt[:, 384 * c + 128 * b:
                                       384 * c + 128 * (b + 1)],
                                    wv_sb[c][:, 512 * n:512 * (n + 1)],
                                    start=(c == 0),
                                    stop=(c == 7 and not bias_v))
                            if bias_v:
                                nc.tensor.matmul(
                                    pv[:], ones_row[:],
                                    bv_sb[:, 512 * n:512 * (n + 1)],
                                    start=False, stop=True)
                            nc.scalar.copy(vs[:, 512 * n:512 * (n + 1)],
                                           pv[:])
                        v_t.append(vs)

                    if V3_ZLATE:
                        emit_vblock(0)
                    if V3_BCAST == "ps":
                        zt = ppz.tile([128, 512], f32, name="z", tag="zzb")
                        z = zt[0:97, :]
                        for b in range(3):
                            nc.tensor.matmul(z, seg_sb[:, 97 * b:
                                                       97 * (b + 1)],
                                             ex_t[b][:],
                                             start=(b == 0), stop=(b == 2))
                        zr_t = []
                        with nc.allow_low_precision(reason="1/Z in bf16"):
                            for sl in range(4):
                                zrx = smp.tile([1, 512], bf16,
                                               name=f"zr{sl}")
                                nc.vector.reciprocal(
                                    zrx[:], zt[32 * sl:32 * sl + 1, :])
                                zr_t.append(zrx)
                        for b in range(3):
                            zbs = smp.tile([128, 512], bf16, name=f"zbs{b}")
                            nc.gpsimd.partition_broadcast(
                                zbs[:], zr_t[3][:], 128)
                            nc.gpsimd.partition_broadcast(
                                zbs[0:TP, :], zr_t[b][:], TP)
                            al = smp.tile([128, 512], bf16, name=f"al{b}")
                            nc.vector.tensor_tensor(al[:], ex_t[b][:],
                                                    zbs[:], MULT)
                            al_t.append(al)
                    else:
                        zt = ppz.tile([128, 512], f32, name="z", tag="zzb")
                        z = zt[0:4, :]
                        for b in range(3):
                            nc.tensor.matmul(z, seg_sb[:, 4 * b:4 * (b + 1)],
                                             ex_t[b][:],
                                             start=(b == 0), stop=(b == 2))
                        zr = smp.tile([4, 512], bf16, name="zr")
                        with nc.allow_low_precision(reason="1/Z in bf16"):
                            nc.vector.reciprocal(zr[:], z)
                        for b in range(3):
                            zb = ppz.tile([128, 512], f32, name="zb",
                                          tag="zzb")
                            nc.tensor.matmul(zb[:], segt_sb[:, 128 * b:
                                                            128 * (b + 1)],
                                             zr[:], start=True, stop=True)
                            al = smp.tile([128, 512], bf16, name=f"al{b}")
                            nc.vector.tensor_tensor(al[:], ex_t[b][:], zb[:],
                                                    MULT)
                            al_t.append(al)

                    # V projection for the remaining blocks
                    for b in range((1 if V3_ZLATE else 0), 3):
                        emit_vblock(b)

                    if stages < 4:
                        osb_stub = lnp.tile([128, H], f32, name="osb")
                        nc.vector.tensor_copy(osb_stub[:, 0:512], al_t[0][:])
                        nc.vector.tensor_copy(osb_stub[:, 512:1024],
                                              v_t[2][:, 0:512])
                        nc.sync.dma_start(out_d[4 * g:4 * (g + 1)],
                                          osb_stub[:])
                        return

                    # attn.V: s0..s2 rows [0:96] of their block; s3 = three
                    # 32-row strips at [96:128] (explicit tile_position), or
                    # repacked into contiguous [96, .] tiles (1 matmul per c)
                    if V3_S3PACK:
                        vs3 = smp.tile([TP, H], bf16, name="vs3")
                        al3 = smp.tile([TP, 512], bf16, name="al3")
                        for b in range(3):
                            nc.vector.tensor_copy(vs3[32 * b:32 * (b + 1), :],
                                                  v_t[b][TP:128, :])
                            nc.vector.tensor_copy(al3[32 * b:32 * (b + 1), :],
                                                  al_t[b][TP:128, :])
                    rt_t = []
                    po_t = []
                    if V3_OINT and stages >= 4:
                        osb = lnp.tile([128, H], f32, name="osb")
                        for n in range(2):
                            po_t.append(ppmm.tile([128, 512], f32,
                                                  name="po", tag="mm"))
                    for c in range(8):
                        pav = ppav.tile([128, 512], f32, name="pav",
                                        tag="av")
                        for sl in range(3):
                            nc.tensor.matmul(
                                pav[:, 64 * sl:64 * (sl + 1)],
                                v_t[sl][0:TP, 128 * c:128 * (c + 1)],
                                al_t[sl][0:TP, 64 * c:64 * (c + 1)],
                                start=True, stop=True)
                        if V3_S3PACK:
                            nc.tensor.matmul(
                                pav[:, 192:256],
                                vs3[:, 128 * c:128 * (c + 1)],
                                al3[:, 64 * c:64 * (c + 1)],
                                start=True, stop=True)
                        else:
                            for b in range(3):
                                nc.tensor.matmul(
                                    pav[:, 192:256],
                                    v_t[b][TP:128, 128 * c:128 * (c + 1)],
                                    al_t[b][TP:128, 64 * c:64 * (c + 1)],
                                    start=(b == 0), stop=(b == 2),
                                    tile_position=(TP, 0))
                        rt = smp.tile([128, 128], bf16, name=f"rt{c}")
                        eng_a = (nc.vector if V3_GATHER in ("dve", "split")
                                 else nc.gpsimd)
                        eng_b = (nc.vector if V3_GATHER == "dve"
                                 else nc.gpsimd)
                        eng_a.tensor_copy(
                            rt[0:64, :].rearrange("p (s q) -> p s q", q=32),
                            pav[0:64, 0:256]
                            .rearrange("p (s q2) -> p s q2", q2=64)[:, :, 0:32])
                        eng_b.tensor_copy(
                            rt[64:128, :].rearrange("p (s q) -> p s q", q=32),
                            pav[64:128, 0:256]
                            .rearrange("p (s q2) -> p s q2", q2=64)[:, :, 32:64])
                        rt_t.append(rt)
                        if V3_OINT:
                            for n in range(2):
                                nc.tensor.matmul(
                                    po_t[n][:], rt[:],
                                    wo_sb[c][:, 512 * n:512 * (n + 1)],
                                    start=(c == 0),
                                    stop=(c == 7 and not bias_o))

                    # O-proj
                    if V3_OINT:
                        for n in range(2):
                            if bias_o:
                                nc.tensor.matmul(
                                    po_t[n][:], ones_row[:],
                                    bo_sb[:, 512 * n:512 * (n + 1)],
                                    start=False, stop=True)
                            nc.scalar.copy(osb[:, 512 * n:512 * (n + 1)],
                                           po_t[n][:])
                    else:
                        osb = lnp.tile([128, H], f32, name="osb")
                        for n in range(2):
                            po = ppmm.tile([128, 512], f32, name="po",
                                           tag="mm")
                            for c in range(8):
                                nc.tensor.matmul(
                                    po[:], rt_t[c][:],
                                    wo_sb[c][:, 512 * n:512 * (n + 1)],
                                    start=(c == 0),
                                    stop=(c == 7 and not bias_o))
                            if bias_o:
                                nc.tensor.matmul(
                                    po[:], ones_row[:],
                                    bo_sb[:, 512 * n:512 * (n + 1)],
                                    start=False, stop=True)
                            nc.scalar.copy(osb[:, 512 * n:512 * (n + 1)],
                                           po[:])

                    # LayerNorm
                    s1 = lnp.tile([128, 1], f32, name="s1")
                    nc.vector.tensor_reduce(s1[:], osb[:], axis=AXX, op=ADD)
                    mean = lnp.tile([128, 1], f32, name="mean")
                    nc.vector.tensor_scalar(mean[:], s1[:], 1.0 / H, None,
                                            MULT)
                    nc.vector.tensor_scalar(osb[:], osb[:], mean[:], None,
                                            SUB)
                    sq = lnp.tile([128, H], f32, name="sq")
                    nc.vector.tensor_tensor(sq[:], osb[:], osb[:], MULT)
                    ssq = lnp.tile([128, 1], f32, name="ssq")
                    nc.vector.tensor_reduce(ssq[:], sq[:], axis=AXX, op=ADD)
                    lnv = lnp.tile([128, 1], f32, name="lnv")
                    nc.scalar.activation(lnv[:], ssq[:], LN_F,
                                         bias=eps_sb[:], scale=1.0 / H)
                    rstd = lnp.tile([128, 1], f32, name="rstd")
                    nc.scalar.activation(rstd[:], lnv[:], EXP, scale=-0.5)
                    nc.vector.tensor_scalar(osb[:], osb[:], rstd[:], None,
                                            MULT)
                    if gamma_beta:
                        nc.vector.tensor_tensor(osb[:], osb[:], gam_sb[:],
                                                MULT)
                        nc.vector.tensor_tensor(osb[:], osb[:], bet_sb[:],
                                                ADD)

                    nc.gpsimd.dma_start(out_d[4 * g:4 * (g + 1)], osb[:])

                def emit_all():
                    for g in range(NG):
                        emit_chunk(g)

                if loop > 1:
                    with tc.For_i(0, loop, 1):
                        emit_all()
                else:
                    emit_all()

    import concourse.bacc as bacc_mod
    _orig_gat = bacc_mod.get_activation_tables
    _pin = {mybir.ActivationFunctionType.Exp, mybir.ActivationFunctionType.Ln,
            mybir.ActivationFunctionType.Copy,
            mybir.ActivationFunctionType.Identity}

    def _gat(arch):
        tables = _orig_gat(arch)
        return {name: (funcs if name == "natural_log_exp_and_others"
                       else funcs - _pin)
                for name, funcs in tables.items()}

    bacc_mod.get_activation_tables = _gat
    try:
        nc.compile()
    finally:
        bacc_mod.get_activation_tables = _orig_gat
    return nc


def _get(loop=1, bias_kq=False, bias_v=False, bias_o=False,
         gamma_beta=False, stages=9, ver=2):
    key = (loop, bias_kq, bias_v, bias_o, gamma_beta, stages,
           ZMODE, IOP_BUFS, SMP_BUFS, LNP_BUFS, MM_BUFS, ver, V3_BCAST,
           V3_PP, V3_GATHER, V3_ZLATE, V3_OINT, V3_S3PACK)
    if key not in _BUILD_CACHE:
        bld = _build3 if ver == 3 else _build
        _BUILD_CACHE[key] = bld(bias_kq=bias_kq, bias_v=bias_v,
                                bias_o=bias_o, gamma_beta=gamma_beta,
                                loop=loop, stages=stages)
    return _BUILD_CACHE[key]


_PREP_CACHE = {}


def _prep_fns():
    """jitted CPU preprocessing (transpose/cast are multithreaded in XLA)."""
    if _PREP_CACHE:
        return _PREP_CACHE
    import jax
    import jax.numpy as jnp

    cpu = jax.devices("cpu")[0]

    def _xkt(x):  # [S*T, H] f32 -> [16, 128, 8, 512] bf16
        x4 = x.reshape(NG, 512, 8, 128)
        return x4.transpose(0, 3, 2, 1).astype(jnp.bfloat16)

    def _qkt(ini_q, Wq, bq, Wk):  # -> [8, 128, HQ] bf16
        q = ini_q @ Wq.T + bq                      # [Q, H]
        qh = q.reshape(Q, HEADS, D)
        qk = jnp.einsum("qhd,hdH->hqH", qh,
                        Wk.reshape(HEADS, D, H)) * np.float32(0.125)
        qkt = qk.reshape(HQ, H).T                  # [H, HQ]
        return qkt.reshape(8, 128, HQ).astype(jnp.bfloat16)

    def _bkq(ini_q, Wq, bq, bk):  # -> [1, HQ] bf16
        q = ini_q @ Wq.T + bq
        qh = q.reshape(Q, HEADS, D)
        t2 = jnp.einsum("qhd,hd->hq", qh,
                        bk.reshape(HEADS, D)) * np.float32(0.125)
        return t2.reshape(1, HQ).astype(jnp.bfloat16)

    def _xkt3(x, mask):  # x [S, T, H] f32, mask [S, T] -> packed X^T bf16
        order = jnp.argsort(mask, axis=-1, stable=True)
        cnt = (mask == 0).sum(-1)
        idx = order[:, :TP]
        valid = jnp.arange(TP)[None, :] < cnt[:, None]
        Xg = jnp.take_along_axis(x, idx[:, :, None], axis=1)  # [S, TP, H]
        Xg = Xg * valid[:, :, None]
        Xg = Xg.reshape(16, 4, TP, H)
        main = Xg[:, 0:3]                          # [16, 3, 96, H]
        s3 = Xg[:, 3].reshape(16, 3, 32, H)        # strips of segment 3
        blocks = jnp.concatenate([main, s3], axis=2)   # [16, 3, 128, H]
        rows = blocks.reshape(16, 384, H)
        xkt = rows.reshape(16, 384, 8, 128).transpose(0, 3, 2, 1)
        return xkt.astype(jnp.bfloat16)

    def _mneg3(mask):  # [S, T] -> [128, 48] exp bias (0 real / -1e4 pad)
        cnt = (mask == 0).sum(-1).reshape(16, 4)
        bias_main = jnp.where(
            jnp.arange(TP)[None, None, :] < cnt[:, 0:3, None], 0.0, -1e4)
        r3 = jnp.arange(3)[None, :, None] * 32 + jnp.arange(32)[None, None, :]
        bias_s3 = jnp.where(r3 < cnt[:, 3, None, None], 0.0, -1e4)
        bias = jnp.concatenate([bias_main, bias_s3], axis=2)  # [16, 3, 128]
        return bias.transpose(2, 0, 1).reshape(128, 48).astype(jnp.float32)

    _PREP_CACHE["xkt"] = jax.jit(_xkt, device=cpu)
    _PREP_CACHE["qkt"] = jax.jit(_qkt, device=cpu)
    _PREP_CACHE["bkq"] = jax.jit(_bkq, device=cpu)
    _PREP_CACHE["xkt3"] = jax.jit(_xkt3, device=cpu)
    _PREP_CACHE["mneg3"] = jax.jit(_mneg3, device=cpu)
    return _PREP_CACHE


def _in_maps(ini_q, ini_k, mask, Wq, bq, Wk, bk, Wv, bv, Wo, bo, gamma, beta):
    import ml_dtypes
    f = np.float32
    bfdt = ml_dtypes.bfloat16
    fns = _prep_fns()

    wvt = np.asarray(Wv, dtype=f).T.astype(bfdt)
    wot = np.asarray(Wo, dtype=f).T.astype(bfdt)
    shared = dict(
        wvt=np.ascontiguousarray(wvt),
        wot=np.ascontiguousarray(wot),
        bvr=np.asarray(bv, dtype=f).reshape(1, H).astype(bfdt),
        bor=np.asarray(bo, dtype=f).reshape(1, H).astype(bfdt),
        gam=np.asarray(gamma, dtype=f).reshape(1, H),
        bet=np.asarray(beta, dtype=f).reshape(1, H),
    )
    ini_q = np.asarray(ini_q, dtype=f)
    ini_k = np.asarray(ini_k, dtype=f)
    mask = np.asarray(mask, dtype=f)
    Wq_, bq_, Wk_, bk_ = (np.asarray(a, dtype=f) for a in (Wq, bq, Wk, bk))
    maps = []
    for b in range(B):
        m = dict(shared)
        m["xkt"] = np.asarray(fns["xkt"](ini_k[b].reshape(ST, H)))
        m["qkt"] = np.asarray(fns["qkt"](ini_q[b], Wq_, bq_, Wk_))
        m["bkq"] = np.asarray(fns["bkq"](ini_q[b], Wq_, bq_, bk_))
        m["mnegt"] = np.ascontiguousarray(mask[b].T * f(-10000.0))
        maps.append(m)
    return maps


def _seg_consts():
    import ml_dtypes
    bfdt = ml_dtypes.bfloat16
    if V3_BCAST == "ps":
        seg = np.zeros((128, 3 * 97), np.float32)
        for b in range(3):
            seg[0:TP, 97 * b + 32 * b] = 1.0
            seg[TP:128, 97 * b + 96] = 1.0
    else:
        seg = np.zeros((128, 12), np.float32)
        for b in range(3):
            seg[0:TP, 4 * b + b] = 1.0
            seg[TP:128, 4 * b + 3] = 1.0
    segt = np.zeros((4, 384), np.float32)
    for b in range(3):
        segt[b, 128 * b:128 * b + TP] = 1.0
        segt[3, 128 * b + TP:128 * (b + 1)] = 1.0
    return seg.astype(bfdt), segt.astype(bfdt)


def _in_maps3(ini_q, ini_k, mask, Wq, bq, Wk, bk, Wv, bv, Wo, bo, gamma,
              beta):
    import ml_dtypes
    f = np.float32
    bfdt = ml_dtypes.bfloat16
    fns = _prep_fns()

    wvt = np.asarray(Wv, dtype=f).T.astype(bfdt)
    wot = np.asarray(Wo, dtype=f).T.astype(bfdt)
    seg, segt = _seg_consts()
    shared = dict(
        wvt=np.ascontiguousarray(wvt),
        wot=np.ascontiguousarray(wot),
        seg3=seg, segt3=segt,
        bvr=np.asarray(bv, dtype=f).reshape(1, H).astype(bfdt),
        bor=np.asarray(bo, dtype=f).reshape(1, H).astype(bfdt),
        gam=np.asarray(gamma, dtype=f).reshape(1, H),
        bet=np.asarray(beta, dtype=f).reshape(1, H),
    )
    ini_q = np.asarray(ini_q, dtype=f)
    ini_k = np.asarray(ini_k, dtype=f)
    mask = np.asarray(mask, dtype=f)
    Wq_, bq_, Wk_, bk_ = (np.asarray(a, dtype=f) for a in (Wq, bq, Wk, bk))
    maps = []
    for b in range(B):
        m = dict(shared)
        m["xkt3"] = np.asarray(fns["xkt3"](ini_k[b], mask[b]))
        m["mneg3"] = np.asarray(fns["mneg3"](mask[b]))
        m["qkt"] = np.asarray(fns["qkt"](ini_q[b], Wq_, bq_, Wk_))
        m["bkq"] = np.asarray(fns["bkq"](ini_q[b], Wq_, bq_, bk_))
        maps.append(m)
    return maps


def run(inputs, loop=1, full_results=False, stages=9, ver=None):
    """Run the SPMD kernel; returns (B, Q, S, H) float32."""
    from concourse.bass_utils import run_bass_kernel_spmd

    if ver is None:
        cnt_max = int((np.asarray(inputs["mask"]) == 0).sum(-1).max())
        ver = 3 if cnt_max <= TP else 2
    flags = dict(
        stages=stages,
        bias_kq=bool(np.any(inputs["bq"]) or np.any(inputs["bk"])),
        bias_v=bool(np.any(inputs["bv"])),
        bias_o=bool(np.any(inputs["bo"])),
        gamma_beta=bool(np.any(np.asarray(inputs["gamma"]) != 1.0)
                        or np.any(inputs["beta"])),
    )
    nc = _get(loop=loop, ver=ver, **flags)
    maps = _in_maps3(**inputs) if ver == 3 else _in_maps(**inputs)
    err = None
    for _ in range(4):
        try:
            res = run_bass_kernel_spmd(nc, maps, list(range(NCORES)))
        except Exception as e:  # transient NRT device errors: retry
            err = e
            import time as _t
            _t.sleep(2.0)
            continue
        if full_results:
            return res
        # device output is s-major [S, Q, H]; transpose back to [Q, S, H]
        out = np.stack([res.results[c]["out"].transpose(1, 0, 2)
                        for c in range(NCORES)], axis=0)
        # transient first-execution corruption has been observed once on
        # this fleet; non-finite output -> re-execute
        if np.isfinite(out).all():
            return out
        err = RuntimeError("non-finite kernel output")
    raise err


def kernel(**inputs):
    return run(inputs, loop=1)



# revision 18
# speedup vs baseline: 1.0309x; 1.0309x over previous
"""Trainium2 Bass kernel for nn_MultiHeadTokenAttention — v3.

v3 (default when every (b,s) has <= 96 unmasked tokens; else falls back to
the dense v2 path below): mask-sparsity packing.  The binary mask kills
~half the t positions (softmax weight exp(-1e4) ~ 0), so the host packs
only the unmasked rows of each segment (b, s), padded to a uniform TP=96,
4 segments per 3 x 128-partition blocks (s0..s2 at rows [0:96] of blocks
0..2, s3 split into three 32-row strips at rows [96:128]).  scores and the
V projection then run on 48 instead of 64 row-blocks (-25% PE columns).
Per-segment softmax sums use 0/1 segment-indicator matmuls (z97 variant
keeps slots at partitions {0,32,64,96}); the 1/Z broadcast back to rows is
a [4,512]-stationary matmul.  attn.V runs per segment as partition-range
matmuls: K=96 at base 0 for s0..s2, and 3 x K=32 at base 96 for s3 (the
96 base requires an explicit tile_position=(96, 0) — the auto path only
allows bases {0,32,64}).  Padding rows enter exp with a -1e4 bias so their
alpha is exactly 0.  Everything else (rt gather, O projection, LayerNorm,
s-major store) is unchanged from v2.

Measured on the 8-core axon fleet: ~0.51-0.56 ms/iter steady-state vs
~0.59-0.62 ms for tuned v2 (and 1.26-1.87 ms for the prior session's
wall-clock-based measurement of the same v2 kernel).  Rel err 5.3e-3.

--- v2 notes (dense path, kept as fallback) ---

Reference computation (per batch element b):
    q = ini_q @ Wq.T + bq                      [Q, H] -> heads [Q, 16, 64]
    k = X @ Wk.T + bk ;  v = X @ Wv.T + bv     (X = ini_k[b] as [S*T, H])
    scores[h,q,s,t] = (q_h . k_h) / 8,  + mask*-1e4, softmax over t
    res[q,s,:] = concat_h(sum_t alpha * v_h)   [Q, S, H]
    res = res @ Wo.T + bo;  LayerNorm(res) * gamma + beta

Sharding: batch-parallel, one batch element per NeuronCore (8 cores, no
collectives).

v2 structure (host + device):
  * Host folds Wk into the queries:  qk[32h+q, :] = scale * q_h @ Wk_h
    so  scoresT[st, hq] = X @ qk^T  -- the K projection is never
    materialized (4.3G MACs instead of 8.9G) and no K^T is needed.
  * Host pre-transposes X to bf16 X^T, tiled [16 chunks][128 p][8 c][512]
    so each chunk's load is one fully-contiguous 1 MB DMA and the device
    does zero transposes (PE transposes were ~20% of baseline PE time).
  * Softmax runs in the transposed layout [t on partitions, hq free]:
    mask enters as the per-partition bias of the exp activation; column
    sums via a ones-stationary matmul; 1/Z broadcast across partitions
    via a K=1 matmul; one DVE multiply -> alphaT (bf16).
  * attn.V: lhsT = V_s [128 t, 128 hd (2 heads)], rhs = alphaT cols of
    the same 2 heads -> out [128 hd, 64]; diagonal 64x32 blocks are the
    valid res^T entries, gathered by 2 strided DVE copies per head-pair
    into rt_c [128 hd, 128 (s,q)] which feeds the O projection as its
    stationary operand unchanged.  LayerNorm as in v1.

All matmuls run bf16 (f32 PSUM accumulate); rel err vs f32 reference is
~2e-3, comfortably under the 2e-2 gate.
"""

import os
import sys

for _p in ("/opt/trn_rl_repo", "/root/.axon_site/_ro/trn_rl_repo"):
    if os.path.isdir(_p) and _p not in sys.path:
        sys.path.insert(0, _p)

import numpy as np

B, Q, S, T, H = 8, 32, 64, 128, 1024
HEADS, D = 16, 64
ST = S * T           # 8192 rows of X per batch element
NCORES = 8
NG = 16              # chunks per core (4 s-values = 512 st rows each)
HQ = HEADS * Q       # 512
EPS = 1e-12

_BUILD_CACHE = {}

# softmax 1/Z plumbing: "mm" = colsum+broadcast via PE matmuls;
# "ar" = gpsimd partition_all_reduce + DVE recip/mult (no PE work, no PSUM)
ZMODE = os.environ.get("KV2_ZMODE", "ar")
IOP_BUFS = int(os.environ.get("KV2_IOP_BUFS", "3"))
V3_BCAST = os.environ.get("KV3_BCAST", "mm")  # "mm" matmul | "ps" pool-bcast
V3_PP = os.environ.get("KV3_PP", "")  # "mm,zzb,av" PSUM ring sizes
V3_GATHER = os.environ.get("KV3_GATHER", "dve")  # dve | pool | split
V3_ZLATE = int(os.environ.get("KV3_ZLATE", "0"))  # 1: z/zb after V block 0
V3_OINT = int(os.environ.get("KV3_OINT", "0"))   # 1: interleave O with attnV
V3_S3PACK = int(os.environ.get("KV3_S3PACK", "1"))  # 1: repack s3 strips (506us vs 527us)
V3_ZSHARE = int(os.environ.get("KV3_ZSHARE", "0"))  # 1: z/zb in the mm ring
V3_VPAIR = int(os.environ.get("KV3_VPAIR", "0"))  # 1: n0/n1 share lhsT
SMP_BUFS = int(os.environ.get("KV2_SMP_BUFS", "2"))
LNP_BUFS = int(os.environ.get("KV2_LNP_BUFS", "2"))
MM_BUFS = int(os.environ.get("KV2_MM_BUFS", "0"))  # 0 -> default
if not MM_BUFS and ZMODE == "ar":
    MM_BUFS = 6  # measured: 590us vs 614us at 5


def _build(bias_kq=False, bias_v=False, bias_o=False, gamma_beta=False,
           loop=1, stages=9):
    """Build + compile the Bass program. Returns the Bacc object."""
    import concourse.mybir as mybir
    from concourse import bacc
    from concourse.tile import TileContext

    f32 = mybir.dt.float32
    bf16 = mybir.dt.bfloat16
    ADD = mybir.AluOpType.add
    SUB = mybir.AluOpType.subtract
    MULT = mybir.AluOpType.mult
    AXX = mybir.AxisListType.X
    EXP = mybir.ActivationFunctionType.Exp
    LN_F = mybir.ActivationFunctionType.Ln
    DIV = mybir.AluOpType.divide
    from concourse import bass_isa

    nc = bacc.Bacc("TRN2", target_bir_lowering=False, debug=False,
                   num_devices=NCORES)

    # X^T bf16, tiled: xkt[g, p, c, j] = X[512 g + j, 128 c + p]
    xkt_d = nc.dram_tensor("xkt", [NG, 128, 8, 512], bf16,
                           kind="ExternalInput")
    # qk^T bf16: qkt[c, p, m] = qk[m, 128 c + p]  (m = 32 h + q)
    qkt_d = nc.dram_tensor("qkt", [8, 128, HQ], bf16, kind="ExternalInput")
    # mask^T * -1e4: mnegt[t, s]
    mnegt_d = nc.dram_tensor("mnegt", [T, S], f32, kind="ExternalInput")
    # Wv^T bf16 rows h cols hd; Wo^T bf16 rows hd cols H
    wvt_d = nc.dram_tensor("wvt", [H, H], bf16, kind="ExternalInput")
    wot_d = nc.dram_tensor("wot", [H, H], bf16, kind="ExternalInput")
    bkq_d = nc.dram_tensor("bkq", [1, HQ], bf16, kind="ExternalInput")
    bv_d = nc.dram_tensor("bvr", [1, H], bf16, kind="ExternalInput")
    bo_d = nc.dram_tensor("bor", [1, H], bf16, kind="ExternalInput")
    gam_d = nc.dram_tensor("gam", [1, H], f32, kind="ExternalInput")
    bet_d = nc.dram_tensor("bet", [1, H], f32, kind="ExternalInput")
    # s-major output: contiguous 512 KB write per chunk (the q-major layout
    # costs 128 scattered 4 KB descriptors per chunk and dominates the
    # critical path); host returns a transposed view.
    out_d = nc.dram_tensor("out", [S, Q, H], f32, kind="ExternalOutput")

    mm_bufs = MM_BUFS or (3 if ZMODE == "mm" else 5)
    with TileContext(nc) as tc:
        with tc.tile_pool(name="wts", bufs=1) as wpool, \
             tc.tile_pool(name="ppmm", bufs=mm_bufs, space="PSUM") as ppmm, \
             tc.tile_pool(name="ppz", bufs=1, space="PSUM") as ppz, \
             tc.tile_pool(name="ppzb", bufs=2, space="PSUM") as ppzb, \
             tc.tile_pool(name="ppav", bufs=2, space="PSUM") as ppav:

            # ---------------- preamble: constants + weights ----------------
            eps_sb = wpool.tile([128, 1], f32, name="eps_sb")
            nc.vector.memset(eps_sb[:], EPS)
            ones_col = wpool.tile([128, 1], bf16, name="ones_col")
            nc.vector.memset(ones_col[:], 1.0)
            ones_row = wpool.tile([1, 128], bf16, name="ones_row")
            nc.vector.memset(ones_row[:], 1.0)
            ones_row_f = wpool.tile([1, 128], f32, name="ones_row_f")
            nc.vector.memset(ones_row_f[:], 1.0)

            mneg_sb = wpool.tile([T, S], f32, name="mneg_sb")
            nc.sync.dma_start(mneg_sb[:], mnegt_d[:])

            qk_sb, wv_sb, wo_sb = [], [], []
            for c in range(8):
                qkc = wpool.tile([128, HQ], bf16, name=f"qk{c}")
                nc.gpsimd.dma_start(qkc[:], qkt_d[c])
                qk_sb.append(qkc)
                wvc = wpool.tile([128, H], bf16, name=f"wv{c}")
                nc.gpsimd.dma_start(wvc[:], wvt_d[128 * c:128 * (c + 1), :])
                wv_sb.append(wvc)
                woc = wpool.tile([128, H], bf16, name=f"wo{c}")
                nc.gpsimd.dma_start(woc[:], wot_d[128 * c:128 * (c + 1), :])
                wo_sb.append(woc)

            if bias_kq:
                bkq_sb = wpool.tile([1, HQ], bf16, name="bkq_sb")
                nc.gpsimd.dma_start(bkq_sb[:], bkq_d[:])
            if bias_v:
                bv_sb = wpool.tile([1, H], bf16, name="bv_sb")
                nc.gpsimd.dma_start(bv_sb[:], bv_d[:])
            if bias_o:
                bo_sb = wpool.tile([1, H], bf16, name="bo_sb")
                nc.gpsimd.dma_start(bo_sb[:], bo_d[:])
            if gamma_beta:
                gam_sb = wpool.tile([128, H], f32, name="gam_sb")
                bet_sb = wpool.tile([128, H], f32, name="bet_sb")
                nc.sync.dma_start(
                    gam_sb[:], gam_d[0, :].partition_broadcast(128))
                nc.sync.dma_start(
                    bet_sb[:], bet_d[0, :].partition_broadcast(128))

            # ---------------- main per-chunk pipeline ----------------
            with tc.tile_pool(name="io", bufs=IOP_BUFS) as iop, \
                 tc.tile_pool(name="sm", bufs=SMP_BUFS) as smp, \
                 tc.tile_pool(name="ln", bufs=LNP_BUFS) as lnp:

                def emit_chunk(g):
                    # 1. load X^T chunk: one contiguous 1 MB DMA.  Issued on
                    # the (otherwise idle) Pool queue so it never queues
                    # behind the out-store on SP.
                    xt = iop.tile([128, 4096], bf16, name="xt")
                    nc.sync.dma_start(
                        xt[:].rearrange("p (c j) -> p c j", c=8), xkt_d[g])

                    ex_t, al_t, v_t = [], [], []
                    for sp in range(4):
                        # 2. scoresT[t, hq] for s = 4g+sp
                        ps = ppmm.tile([128, 512], f32, name="ps", tag="mm")
                        for c in range(8):
                            nc.tensor.matmul(
                                ps[:], xt[:, 512 * c + 128 * sp:
                                          512 * c + 128 * (sp + 1)],
                                qk_sb[c][:],
                                start=(c == 0),
                                stop=(c == 7 and not bias_kq))
                        if bias_kq:
                            nc.tensor.matmul(ps[:], ones_row[:], bkq_sb[:],
                                             start=False, stop=True)
                        # 3. exp(scoresT + mask_col) -> bf16, mask via bias
                        ex = smp.tile([128, 512], bf16, name=f"ex{sp}")
                        nc.scalar.activation(
                            ex[:], ps[:], EXP,
                            bias=mneg_sb[:, 4 * g + sp:4 * g + sp + 1])
                        ex_t.append(ex)
                        al = smp.tile([128, 512], bf16, name=f"al{sp}")
                        if ZMODE == "ar":
                            # 4+5. Z bcast via gpsimd all-reduce;
                            # al = ex * (1/Z)  (DVE divide is not valid ISA)
                            zsb = smp.tile([128, 512], f32, name="zsb",
                                           tag="zsb", bufs=2)
                            nc.gpsimd.partition_all_reduce(
                                zsb[:], ex[:], 128, bass_isa.ReduceOp.add)
                            zrb = smp.tile([128, 512], f32, name="zrb",
                                           tag="zrb", bufs=2)
                            nc.vector.reciprocal(zrb[:], zsb[:])
                            nc.vector.tensor_tensor(al[:], ex[:], zrb[:],
                                                    MULT)
                        else:
                            # 4. Z[hq] colsums via ones-stationary matmul
                            z = ppz.tile([1, 512], f32, name="z", tag="z")
                            nc.tensor.matmul(z[:], ones_col[:], ex[:],
                                             start=True, stop=True)
                            zr = smp.tile([1, 512], f32, name="zr",
                                          tag="zr", bufs=2)
                            nc.vector.reciprocal(zr[:], z[:])
                            # 5. bcast 1/Z across partitions via K=1 matmul
                            zb = ppzb.tile([128, 512], f32, name="zb",
                                           tag="zb")
                            nc.tensor.matmul(zb[:], ones_row_f[:], zr[:],
                                             start=True, stop=True)
                            nc.vector.tensor_tensor(al[:], ex[:], zb[:],
                                                    MULT)
                        al_t.append(al)
                        # 6. V_s[t, hd] natural
                        vs = smp.tile([128, H], bf16, name=f"v{sp}")
                        for n in range(2):
                            pv = ppmm.tile([128, 512], f32, name="pv",
                                           tag="mm")
                            for c in range(8):
                                nc.tensor.matmul(
                                    pv[:],
                                    xt[:, 512 * c + 128 * sp:
                                       512 * c + 128 * (sp + 1)],
                                    wv_sb[c][:, 512 * n:512 * (n + 1)],
                                    start=(c == 0),
                                    stop=(c == 7 and not bias_v))
                            if bias_v:
                                nc.tensor.matmul(
                                    pv[:], ones_row[:],
                                    bv_sb[:, 512 * n:512 * (n + 1)],
                                    start=False, stop=True)
                            nc.scalar.copy(vs[:, 512 * n:512 * (n + 1)],
                                           pv[:])
                        v_t.append(vs)

                    if stages < 4:
                        osb_stub = lnp.tile([128, H], f32, name="osb")
                        nc.vector.tensor_copy(osb_stub[:, 0:512], al_t[0][:])
                        nc.vector.tensor_copy(osb_stub[:, 512:1024],
                                              v_t[3][:, 0:512])
                        nc.sync.dma_start(out_d[4 * g:4 * (g + 1)],
                                          osb_stub[:])
                        return

                    # 7. attn.V -> rt_c [128 hd (2 heads), 128 (s, q)]
                    # pav is a full PSUM bank: a half-bank tile would share
                    # its physical bank with the pool's other rotation buf,
                    # and PE-write + DVE-read of one bank is a fatal HW
                    # PSUM collision (not modeled by CoreSim).
                    rt_t = []
                    for c in range(8):
                        pav = ppav.tile([128, 512], f32, name="pav",
                                        tag="av")
                        for sp in range(4):
                            nc.tensor.matmul(
                                pav[:, 64 * sp:64 * (sp + 1)],
                                v_t[sp][:, 128 * c:128 * (c + 1)],
                                al_t[sp][:, 64 * c:64 * (c + 1)],
                                start=True, stop=True)
                        rt = smp.tile([128, 128], bf16, name=f"rt{c}")
                        eng_a = (nc.vector if V3_GATHER in ("dve", "split")
                                 else nc.gpsimd)
                        eng_b = (nc.vector if V3_GATHER == "dve"
                                 else nc.gpsimd)
                        eng_a.tensor_copy(
                            rt[0:64, :].rearrange("p (s q) -> p s q", q=32),
                            pav[0:64, 0:256]
                            .rearrange("p (s q2) -> p s q2", q2=64)[:, :, 0:32])
                        eng_b.tensor_copy(
                            rt[64:128, :].rearrange("p (s q) -> p s q", q=32),
                            pav[64:128, 0:256]
                            .rearrange("p (s q2) -> p s q2", q2=64)[:, :, 32:64])
                        rt_t.append(rt)

                    if stages < 6:
                        osb_stub = lnp.tile([128, H], f32, name="osb")
                        nc.vector.tensor_copy(osb_stub[:, 0:64],
                                              rt_t[0][:].bitcast(f32))
                        nc.vector.tensor_copy(osb_stub[:, 64:128],
                                              rt_t[7][:].bitcast(f32))
                        nc.vector.tensor_copy(osb_stub[:, 128:256],
                                              osb_stub[:, 0:128])
                        nc.vector.tensor_copy(osb_stub[:, 256:512],
                                              osb_stub[:, 0:256])
                        nc.vector.tensor_copy(osb_stub[:, 512:1024],
                                              osb_stub[:, 0:512])
                        nc.sync.dma_start(out_d[4 * g:4 * (g + 1)],
                                          osb_stub[:])
                        return

                    # 8. O-proj: rows (s, q) on partitions, H on free
                    osb = lnp.tile([128, H], f32, name="osb")
                    for n in range(2):
                        po = ppmm.tile([128, 512], f32, name="po", tag="mm")
                        for c in range(8):
                            nc.tensor.matmul(
                                po[:], rt_t[c][:],
                                wo_sb[c][:, 512 * n:512 * (n + 1)],
                                start=(c == 0),
                                stop=(c == 7 and not bias_o))
                        if bias_o:
                            nc.tensor.matmul(
                                po[:], ones_row[:],
                                bo_sb[:, 512 * n:512 * (n + 1)],
                                start=False, stop=True)
                        nc.scalar.copy(osb[:, 512 * n:512 * (n + 1)], po[:])

                    # 9. LayerNorm over H (in place on osb).  rstd via the
                    # DVE pow ALU op, so ACT only ever needs exp+copy (one
                    # act-table load, hoisted out of the loop).
                    s1 = lnp.tile([128, 1], f32, name="s1")
                    nc.vector.tensor_reduce(s1[:], osb[:], axis=AXX, op=ADD)
                    mean = lnp.tile([128, 1], f32, name="mean")
                    nc.vector.tensor_scalar(mean[:], s1[:], 1.0 / H, None,
                                            MULT)
                    nc.vector.tensor_scalar(osb[:], osb[:], mean[:], None,
                                            SUB)
                    sq = lnp.tile([128, H], f32, name="sq")
                    nc.vector.tensor_tensor(sq[:], osb[:], osb[:], MULT)
                    ssq = lnp.tile([128, 1], f32, name="ssq")
                    nc.vector.tensor_reduce(ssq[:], sq[:], axis=AXX, op=ADD)
                    # ln(ssq/H + eps) via the activation's scale+bias, then
                    # rstd = exp(-0.5 ln(var+eps)); both funcs live in the
                    # pinned act table set.
                    lnv = lnp.tile([128, 1], f32, name="lnv")
                    nc.scalar.activation(lnv[:], ssq[:], LN_F,
                                         bias=eps_sb[:], scale=1.0 / H)
                    rstd = lnp.tile([128, 1], f32, name="rstd")
                    nc.scalar.activation(rstd[:], lnv[:], EXP, scale=-0.5)
                    nc.vector.tensor_scalar(osb[:], osb[:], rstd[:], None,
                                            MULT)
                    if gamma_beta:
                        nc.vector.tensor_tensor(osb[:], osb[:], gam_sb[:],
                                                MULT)
                        nc.vector.tensor_tensor(osb[:], osb[:], bet_sb[:],
                                                ADD)

                    # 10. out[4g:4g+4, :, :] <- rows (s-major, q); fully
                    # contiguous 512 KB store.  Issued via SWDGE (Pool
                    # queue) so the next chunk's xt load on SP never queues
                    # behind it.
                    nc.gpsimd.dma_start(out_d[4 * g:4 * (g + 1)], osb[:])

                def emit_all():
                    for g in range(NG):
                        emit_chunk(g)

                if loop > 1:
                    with tc.For_i(0, loop, 1):
                        emit_all()
                else:
                    emit_all()

    # Pin exp/ln/copy activations to the one act-table set that holds all
    # of them ("natural_log_exp_and_others") so the table load is emitted
    # once and hoisted out of the loop instead of swapping every chunk.
    import concourse.bacc as bacc_mod
    _orig_gat = bacc_mod.get_activation_tables
    _pin = {mybir.ActivationFunctionType.Exp, mybir.ActivationFunctionType.Ln,
            mybir.ActivationFunctionType.Copy,
            mybir.ActivationFunctionType.Identity}

    def _gat(arch):
        tables = _orig_gat(arch)
        return {name: (funcs if name == "natural_log_exp_and_others"
                       else funcs - _pin)
                for name, funcs in tables.items()}

    bacc_mod.get_activation_tables = _gat
    try:
        nc.compile()
    finally:
        bacc_mod.get_activation_tables = _orig_gat
    return nc




# ---------------------------------------------------------------------------
# v3: mask-sparsity packing.  Each segment (b, s) keeps only its unmasked
# t-rows (plus zero padding) at uniform length TP=96 >= max unmasked count.
# Groups of 4 segments pack into 3 x 128-row blocks: s0,s1,s2 occupy rows
# [0:96] of blocks 0,1,2; s3 is split into three 32-row strips at rows
# [96:128] (PE matmul reaches base partition 96 via explicit tile_position).
# scores and the V projection then run on 48 instead of 64 row-blocks.
# Softmax Z per segment via segment-indicator matmuls:
#   z[4, 512]  = sum_b seg_b^T @ ex_b      (seg_b [128,4] 0/1 host constant)
#   zb_b[128, 512] = segT_b @ recip(z)     (broadcast back to rows)
# ---------------------------------------------------------------------------
TP = 96          # packed tokens per segment
NBLK = 48        # 16 groups x 3 blocks


def _build3(bias_kq=False, bias_v=False, bias_o=False, gamma_beta=False,
            loop=1, stages=9):
    import concourse.mybir as mybir
    from concourse import bacc
    from concourse.tile import TileContext

    f32 = mybir.dt.float32
    bf16 = mybir.dt.bfloat16
    ADD = mybir.AluOpType.add
    SUB = mybir.AluOpType.subtract
    MULT = mybir.AluOpType.mult
    AXX = mybir.AxisListType.X
    EXP = mybir.ActivationFunctionType.Exp
    LN_F = mybir.ActivationFunctionType.Ln

    nc = bacc.Bacc("TRN2", target_bir_lowering=False, debug=False,
                   num_devices=NCORES)

    # packed X^T bf16: xkt3[g, p, c, j] = Xp[g, j, 128c + p], j in [0, 384)
    xkt_d = nc.dram_tensor("xkt3", [16, 128, 8, 384], bf16,
                           kind="ExternalInput")
    qkt_d = nc.dram_tensor("qkt", [8, 128, HQ], bf16, kind="ExternalInput")
    # exp bias per (partition, block): 0 for real rows, -1e4 for padding
    mneg_d = nc.dram_tensor("mneg3", [128, NBLK], f32, kind="ExternalInput")
    # seg3 one-hot segment indicators; "ps" variant pads the segment slots
    # to partitions {0,32,64,96} of a [97, 512] PSUM tile so recip reads are
    # 32-aligned and gpsimd broadcast inputs sit at partition 0
    segw = 97 if V3_BCAST == "ps" else 4
    seg_d = nc.dram_tensor("seg3", [128, 3 * segw], bf16,
                           kind="ExternalInput")
    segt_d = nc.dram_tensor("segt3", [4, 384], bf16, kind="ExternalInput")
    wvt_d = nc.dram_tensor("wvt", [H, H], bf16, kind="ExternalInput")
    wot_d = nc.dram_tensor("wot", [H, H], bf16, kind="ExternalInput")
    bkq_d = nc.dram_tensor("bkq", [1, HQ], bf16, kind="ExternalInput")
    bv_d = nc.dram_tensor("bvr", [1, H], bf16, kind="ExternalInput")
    bo_d = nc.dram_tensor("bor", [1, H], bf16, kind="ExternalInput")
    gam_d = nc.dram_tensor("gam", [1, H], f32, kind="ExternalInput")
    bet_d = nc.dram_tensor("bet", [1, H], f32, kind="ExternalInput")
    out_d = nc.dram_tensor("out", [S, Q, H], f32, kind="ExternalOutput")

    if V3_PP:
        mm_bufs, z_bufs, av_bufs = (int(x) for x in V3_PP.split(","))
    elif V3_ZSHARE:
        mm_bufs, z_bufs, av_bufs = 6, 1, 2
    else:
        mm_bufs = 5 if V3_BCAST == "ps" else 4
        z_bufs = 1 if V3_BCAST == "ps" else 2
        av_bufs = 2
    with TileContext(nc) as tc:
        with tc.tile_pool(name="wts", bufs=1) as wpool, \
             tc.tile_pool(name="ppmm", bufs=mm_bufs, space="PSUM") as ppmm, \
             tc.tile_pool(name="ppz", bufs=z_bufs, space="PSUM") as ppz, \
             tc.tile_pool(name="ppav", bufs=av_bufs, space="PSUM") as ppav:

            eps_sb = wpool.tile([128, 1], f32, name="eps_sb")
            nc.vector.memset(eps_sb[:], EPS)
            ones_row = wpool.tile([1, 128], bf16, name="ones_row")
            nc.vector.memset(ones_row[:], 1.0)

            mneg_sb = wpool.tile([128, NBLK], f32, name="mneg_sb")
            nc.sync.dma_start(mneg_sb[:], mneg_d[:])
            seg_sb = wpool.tile([128, 3 * segw], bf16, name="seg_sb")
            nc.sync.dma_start(seg_sb[:], seg_d[:])
            segt_sb = wpool.tile([4, 384], bf16, name="segt_sb")
            nc.sync.dma_start(segt_sb[:], segt_d[:])

            qk_sb, wv_sb, wo_sb = [], [], []
            for c in range(8):
                qkc = wpool.tile([128, HQ], bf16, name=f"qk{c}")
                nc.gpsimd.dma_start(qkc[:], qkt_d[c])
                qk_sb.append(qkc)
                wvc = wpool.tile([128, H], bf16, name=f"wv{c}")
                nc.gpsimd.dma_start(wvc[:], wvt_d[128 * c:128 * (c + 1), :])
                wv_sb.append(wvc)
                woc = wpool.tile([128, H], bf16, name=f"wo{c}")
                nc.gpsimd.dma_start(woc[:], wot_d[128 * c:128 * (c + 1), :])
                wo_sb.append(woc)

            if bias_kq:
                bkq_sb = wpool.tile([1, HQ], bf16, name="bkq_sb")
                nc.gpsimd.dma_start(bkq_sb[:], bkq_d[:])
            if bias_v:
                bv_sb = wpool.tile([1, H], bf16, name="bv_sb")
                nc.gpsimd.dma_start(bv_sb[:], bv_d[:])
            if bias_o:
                bo_sb = wpool.tile([1, H], bf16, name="bo_sb")
                nc.gpsimd.dma_start(bo_sb[:], bo_d[:])
            if gamma_beta:
                gam_sb = wpool.tile([128, H], f32, name="gam_sb")
                bet_sb = wpool.tile([128, H], f32, name="bet_sb")
                nc.sync.dma_start(
                    gam_sb[:], gam_d[0, :].partition_broadcast(128))
                nc.sync.dma_start(
                    bet_sb[:], bet_d[0, :].partition_broadcast(128))

            with tc.tile_pool(name="io", bufs=3) as iop, \
                 tc.tile_pool(name="sm", bufs=2) as smp, \
                 tc.tile_pool(name="ln", bufs=2) as lnp:

                def emit_chunk(g):
                    xt = iop.tile([128, 3072], bf16, name="xt")
                    nc.sync.dma_start(
                        xt[:].rearrange("p (c j) -> p c j", c=8), xkt_d[g])

                    # scores + exp per block
                    ex_t = []
                    for b in range(3):
                        ps = ppmm.tile([128, 512], f32, name="ps", tag="mm")
                        for c in range(8):
                            nc.tensor.matmul(
                                ps[:], xt[:, 384 * c + 128 * b:
                                          384 * c + 128 * (b + 1)],
                                qk_sb[c][:],
                                start=(c == 0),
                                stop=(c == 7 and not bias_kq))
                        if bias_kq:
                            nc.tensor.matmul(ps[:], ones_row[:], bkq_sb[:],
                                             start=False, stop=True)
                        ex = smp.tile([128, 512], bf16, name=f"ex{b}")
                        nc.scalar.activation(
                            ex[:], ps[:], EXP,
                            bias=mneg_sb[:, 3 * g + b:3 * g + b + 1])
                        ex_t.append(ex)

                    # Z per segment via indicator matmuls, then broadcast
                    al_t = []
                    v_t = []

                    def emit_vblock(b):
                        vs = smp.tile([128, H], bf16, name=f"v{b}")
                        if V3_VPAIR:
                            pvs = [ppmm.tile([128, 512], f32, name="pv",
                                             tag="mm") for _ in range(2)]
                            for c in range(8):
                                for n in range(2):
                                    nc.tensor.matmul(
                                        pvs[n][:],
                                        xt[:, 384 * c + 128 * b:
                                           384 * c + 128 * (b + 1)],
                                        wv_sb[c][:, 512 * n:512 * (n + 1)],
                                        start=(c == 0),
                                        stop=(c == 7 and not bias_v))
                            for n in range(2):
                                if bias_v:
                                    nc.tensor.matmul(
                                        pvs[n][:], ones_row[:],
                                        bv_sb[:, 512 * n:512 * (n + 1)],
                                        start=False, stop=True)
                                nc.scalar.copy(vs[:, 512 * n:512 * (n + 1)],
                                               pvs[n][:])
                            v_t.append(vs)
                            return
                        for n in range(2):
                            pv = ppmm.tile([128, 512], f32, name="pv",
                                           tag="mm")
                            for c in range(8):
                                nc.tensor.matmul(
                                    pv[:],
                                    xt[:, 384 * c + 128 * b:
                                       384 * c + 128 * (b + 1)],
                                    wv_sb[c][:, 512 * n:512 * (n + 1)],
                                    start=(c == 0),
                                    stop=(c == 7 and not bias_v))
                            if bias_v:
                                nc.tensor.matmul(
                                    pv[:], ones_row[:],
                                    bv_sb[:, 512 * n:512 * (n + 1)],
                                    start=False, stop=True)
                            nc.scalar.copy(vs[:, 512 * n:512 * (n + 1)],
                                           pv[:])
                        v_t.append(vs)

                    if V3_ZLATE:
                        emit_vblock(0)
                    if V3_BCAST == "ps":
                        zt = ppz.tile([128, 512], f32, name="z", tag="zzb")
                        z = zt[0:97, :]
                        for b in range(3):
                            nc.tensor.matmul(z, seg_sb[:, 97 * b:
                                                       97 * (b + 1)],
                                             ex_t[b][:],
                                             start=(b == 0), stop=(b == 2))
                        zr_t = []
                        with nc.allow_low_precision(reason="1/Z in bf16"):
                            for sl in range(4):
                                zrx = smp.tile([1, 512], bf16,
                                               name=f"zr{sl}")
                                nc.vector.reciprocal(
                                    zrx[:], zt[32 * sl:32 * sl + 1, :])
                                zr_t.append(zrx)
                        for b in range(3):
                            zbs = smp.tile([128, 512], bf16, name=f"zbs{b}")
                            nc.gpsimd.partition_broadcast(
                                zbs[:], zr_t[3][:], 128)
                            nc.gpsimd.partition_broadcast(
                                zbs[0:TP, :], zr_t[b][:], TP)
                            al = smp.tile([128, 512], bf16, name=f"al{b}")
                            nc.vector.tensor_tensor(al[:], ex_t[b][:],
                                                    zbs[:], MULT)
                            al_t.append(al)
                    else:
                        zpool = ppmm if V3_ZSHARE else ppz
                        ztag = "mm" if V3_ZSHARE else "zzb"
                        zt = zpool.tile([128, 512], f32, name="z", tag=ztag)
                        z = zt[0:4, :]
                        for b in range(3):
                            nc.tensor.matmul(z, seg_sb[:, 4 * b:4 * (b + 1)],
                                             ex_t[b][:],
                                             start=(b == 0), stop=(b == 2))
                        zr = smp.tile([4, 512], bf16, name="zr")
                        with nc.allow_low_precision(reason="1/Z in bf16"):
                            nc.vector.reciprocal(zr[:], z)
                        for b in range(3):
                            zb = zpool.tile([128, 512], f32, name="zb",
                                            tag=ztag)
                            nc.tensor.matmul(zb[:], segt_sb[:, 128 * b:
                                                            128 * (b + 1)],
                                             zr[:], start=True, stop=True)
                            al = smp.tile([128, 512], bf16, name=f"al{b}")
                            nc.vector.tensor_tensor(al[:], ex_t[b][:], zb[:],
                                                    MULT)
                            al_t.append(al)

                    # V projection for the remaining blocks
                    for b in range((1 if V3_ZLATE else 0), 3):
                        emit_vblock(b)

                    if stages < 4:
                        osb_stub = lnp.tile([128, H], f32, name="osb")
                        nc.vector.tensor_copy(osb_stub[:, 0:512], al_t[0][:])
                        nc.vector.tensor_copy(osb_stub[:, 512:1024],
                                              v_t[2][:, 0:512])
                        nc.sync.dma_start(out_d[4 * g:4 * (g + 1)],
                                          osb_stub[:])
                        return

                    # attn.V: s0..s2 rows [0:96] of their block; s3 = three
                    # 32-row strips at [96:128] (explicit tile_position), or
                    # repacked into contiguous [96, .] tiles (1 matmul per c)
                    if V3_S3PACK:
                        vs3 = smp.tile([TP, H], bf16, name="vs3")
                        al3 = smp.tile([TP, 512], bf16, name="al3")
                        for b in range(3):
                            nc.vector.tensor_copy(vs3[32 * b:32 * (b + 1), :],
                                                  v_t[b][TP:128, :])
                            nc.vector.tensor_copy(al3[32 * b:32 * (b + 1), :],
                                                  al_t[b][TP:128, :])
                    rt_t = []
                    po_t = []
                    if V3_OINT and stages >= 4:
                        osb = lnp.tile([128, H], f32, name="osb")
                        for n in range(2):
                            po_t.append(ppmm.tile([128, 512], f32,
                                                  name="po", tag="mm"))
                    for c in range(8):
                        pav = ppav.tile([128, 512], f32, name="pav",
                                        tag="av")
                        for sl in range(3):
                            nc.tensor.matmul(
                                pav[:, 64 * sl:64 * (sl + 1)],
                                v_t[sl][0:TP, 128 * c:128 * (c + 1)],
                                al_t[sl][0:TP, 64 * c:64 * (c + 1)],
                                start=True, stop=True)
                        if V3_S3PACK:
                            nc.tensor.matmul(
                                pav[:, 192:256],
                                vs3[:, 128 * c:128 * (c + 1)],
                                al3[:, 64 * c:64 * (c + 1)],
                                start=True, stop=True)
                        else:
                            for b in range(3):
                                nc.tensor.matmul(
                                    pav[:, 192:256],
                                    v_t[b][TP:128, 128 * c:128 * (c + 1)],
                                    al_t[b][TP:128, 64 * c:64 * (c + 1)],
                                    start=(b == 0), stop=(b == 2),
                                    tile_position=(TP, 0))
                        rt = smp.tile([128, 128], bf16, name=f"rt{c}")
                        eng_a = (nc.vector if V3_GATHER in ("dve", "split")
                                 else nc.gpsimd)
                        eng_b = (nc.vector if V3_GATHER == "dve"
                                 else nc.gpsimd)
                        eng_a.tensor_copy(
                            rt[0:64, :].rearrange("p (s q) -> p s q", q=32),
                            pav[0:64, 0:256]
                            .rearrange("p (s q2) -> p s q2", q2=64)[:, :, 0:32])
                        eng_b.tensor_copy(
                            rt[64:128, :].rearrange("p (s q) -> p s q", q=32),
                            pav[64:128, 0:256]
                            .rearrange("p (s q2) -> p s q2", q2=64)[:, :, 32:64])
                        rt_t.append(rt)
                        if V3_OINT:
                            for n in range(2):
                                nc.tensor.matmul(
                                    po_t[n][:], rt[:],
                                    wo_sb[c][:, 512 * n:512 * (n + 1)],
                                    start=(c == 0),
                                    stop=(c == 7 and not bias_o))

                    # O-proj
                    if V3_OINT:
                        for n in range(2):
                            if bias_o:
                                nc.tensor.matmul(
                                    po_t[n][:], ones_row[:],
                                    bo_sb[:, 512 * n:512 * (n + 1)],
                                    start=False, stop=True)
                            nc.scalar.copy(osb[:, 512 * n:512 * (n + 1)],
                                           po_t[n][:])
                    else:
                        osb = lnp.tile([128, H], f32, name="osb")
                        for n in range(2):
                            po = ppmm.tile([128, 512], f32, name="po",
                                           tag="mm")
                            for c in range(8):
                                nc.tensor.matmul(
                                    po[:], rt_t[c][:],
                                    wo_sb[c][:, 512 * n:512 * (n + 1)],
                                    start=(c == 0),
                                    stop=(c == 7 and not bias_o))
                            if bias_o:
                                nc.tensor.matmul(
                                    po[:], ones_row[:],
                                    bo_sb[:, 512 * n:512 * (n + 1)],
                                    start=False, stop=True)
                            nc.scalar.copy(osb[:, 512 * n:512 * (n + 1)],
                                           po[:])

                    # LayerNorm
                    s1 = lnp.tile([128, 1], f32, name="s1")
                    nc.vector.tensor_reduce(s1[:], osb[:], axis=AXX, op=ADD)
                    mean = lnp.tile([128, 1], f32, name="mean")
                    nc.vector.tensor_scalar(mean[:], s1[:], 1.0 / H, None,
                                            MULT)
                    nc.vector.tensor_scalar(osb[:], osb[:], mean[:], None,
                                            SUB)
                    sq = lnp.tile([128, H], f32, name="sq")
                    nc.vector.tensor_tensor(sq[:], osb[:], osb[:], MULT)
                    ssq = lnp.tile([128, 1], f32, name="ssq")
                    nc.vector.tensor_reduce(ssq[:], sq[:], axis=AXX, op=ADD)
                    lnv = lnp.tile([128, 1], f32, name="lnv")
                    nc.scalar.activation(lnv[:], ssq[:], LN_F,
                                         bias=eps_sb[:], scale=1.0 / H)
                    rstd = lnp.tile([128, 1], f32, name="rstd")
                    nc.scalar.activation(rstd[:], lnv[:], EXP, scale=-0.5)
                    nc.vector.tensor_scalar(osb[:], osb[:], rstd[:], None,
                                            MULT)
                    if gamma_beta:
                        nc.vector.tensor_tensor(osb[:], osb[:], gam_sb[:],
                                                MULT)
                        nc.vector.tensor_tensor(osb[:], osb[:], bet_sb[:],
                                                ADD)

                    nc.gpsimd.dma_start(out_d[4 * g:4 * (g + 1)], osb[:])

                def emit_all():
                    for g in range(NG):
                        emit_chunk(g)

                if loop > 1:
                    with tc.For_i(0, loop, 1):
                        emit_all()
                else:
                    emit_all()

    import concourse.bacc as bacc_mod
    _orig_gat = bacc_mod.get_activation_tables
    _pin = {mybir.ActivationFunctionType.Exp, mybir.ActivationFunctionType.Ln,
            mybir.ActivationFunctionType.Copy,
            mybir.ActivationFunctionType.Identity}

    def _gat(arch):
        tables = _orig_gat(arch)
        return {name: (funcs if name == "natural_log_exp_and_others"
                       else funcs - _pin)
                for name, funcs in tables.items()}

    bacc_mod.get_activation_tables = _gat
    try:
        nc.compile()
    finally:
        bacc_mod.get_activation_tables = _orig_gat
    return nc


def _get(loop=1, bias_kq=False, bias_v=False, bias_o=False,
         gamma_beta=False, stages=9, ver=2):
    key = (loop, bias_kq, bias_v, bias_o, gamma_beta, stages,
           ZMODE, IOP_BUFS, SMP_BUFS, LNP_BUFS, MM_BUFS, ver, V3_BCAST,
           V3_PP, V3_GATHER, V3_ZLATE, V3_OINT, V3_S3PACK, V3_ZSHARE,
           V3_VPAIR)
    if key not in _BUILD_CACHE:
        bld = _build3 if ver == 3 else _build
        _BUILD_CACHE[key] = bld(bias_kq=bias_kq, bias_v=bias_v,
                                bias_o=bias_o, gamma_beta=gamma_beta,
                                loop=loop, stages=stages)
    return _BUILD_CACHE[key]


_PREP_CACHE = {}


def _prep_fns():
    """jitted CPU preprocessing (transpose/cast are multithreaded in XLA)."""
    if _PREP_CACHE:
        return _PREP_CACHE
    import jax
    import jax.numpy as jnp

    cpu = jax.devices("cpu")[0]

    def _xkt(x):  # [S*T, H] f32 -> [16, 128, 8, 512] bf16
        x4 = x.reshape(NG, 512, 8, 128)
        return x4.transpose(0, 3, 2, 1).astype(jnp.bfloat16)

    def _qkt(ini_q, Wq, bq, Wk):  # -> [8, 128, HQ] bf16
        q = ini_q @ Wq.T + bq                      # [Q, H]
        qh = q.reshape(Q, HEADS, D)
        qk = jnp.einsum("qhd,hdH->hqH", qh,
                        Wk.reshape(HEADS, D, H)) * np.float32(0.125)
        qkt = qk.reshape(HQ, H).T                  # [H, HQ]
        return qkt.reshape(8, 128, HQ).astype(jnp.bfloat16)

    def _bkq(ini_q, Wq, bq, bk):  # -> [1, HQ] bf16
        q = ini_q @ Wq.T + bq
        qh = q.reshape(Q, HEADS, D)
        t2 = jnp.einsum("qhd,hd->hq", qh,
                        bk.reshape(HEADS, D)) * np.float32(0.125)
        return t2.reshape(1, HQ).astype(jnp.bfloat16)

    def _xkt3(x, mask):  # x [S, T, H] f32, mask [S, T] -> packed X^T bf16
        order = jnp.argsort(mask, axis=-1, stable=True)
        cnt = (mask == 0).sum(-1)
        idx = order[:, :TP]
        valid = jnp.arange(TP)[None, :] < cnt[:, None]
        Xg = jnp.take_along_axis(x, idx[:, :, None], axis=1)  # [S, TP, H]
        Xg = Xg * valid[:, :, None]
        Xg = Xg.reshape(16, 4, TP, H)
        main = Xg[:, 0:3]                          # [16, 3, 96, H]
        s3 = Xg[:, 3].reshape(16, 3, 32, H)        # strips of segment 3
        blocks = jnp.concatenate([main, s3], axis=2)   # [16, 3, 128, H]
        rows = blocks.reshape(16, 384, H)
        xkt = rows.reshape(16, 384, 8, 128).transpose(0, 3, 2, 1)
        return xkt.astype(jnp.bfloat16)

    def _mneg3(mask):  # [S, T] -> [128, 48] exp bias (0 real / -1e4 pad)
        cnt = (mask == 0).sum(-1).reshape(16, 4)
        bias_main = jnp.where(
            jnp.arange(TP)[None, None, :] < cnt[:, 0:3, None], 0.0, -1e4)
        r3 = jnp.arange(3)[None, :, None] * 32 + jnp.arange(32)[None, None, :]
        bias_s3 = jnp.where(r3 < cnt[:, 3, None, None], 0.0, -1e4)
        bias = jnp.concatenate([bias_main, bias_s3], axis=2)  # [16, 3, 128]
        return bias.transpose(2, 0, 1).reshape(128, 48).astype(jnp.float32)

    _PREP_CACHE["xkt"] = jax.jit(_xkt, device=cpu)
    _PREP_CACHE["qkt"] = jax.jit(_qkt, device=cpu)
    _PREP_CACHE["bkq"] = jax.jit(_bkq, device=cpu)
    _PREP_CACHE["xkt3"] = jax.jit(_xkt3, device=cpu)
    _PREP_CACHE["mneg3"] = jax.jit(_mneg3, device=cpu)
    return _PREP_CACHE


def _in_maps(ini_q, ini_k, mask, Wq, bq, Wk, bk, Wv, bv, Wo, bo, gamma, beta):
    import ml_dtypes
    f = np.float32
    bfdt = ml_dtypes.bfloat16
    fns = _prep_fns()

    wvt = np.asarray(Wv, dtype=f).T.astype(bfdt)
    wot = np.asarray(Wo, dtype=f).T.astype(bfdt)
    shared = dict(
        wvt=np.ascontiguousarray(wvt),
        wot=np.ascontiguousarray(wot),
        bvr=np.asarray(bv, dtype=f).reshape(1, H).astype(bfdt),
        bor=np.asarray(bo, dtype=f).reshape(1, H).astype(bfdt),
        gam=np.asarray(gamma, dtype=f).reshape(1, H),
        bet=np.asarray(beta, dtype=f).reshape(1, H),
    )
    ini_q = np.asarray(ini_q, dtype=f)
    ini_k = np.asarray(ini_k, dtype=f)
    mask = np.asarray(mask, dtype=f)
    Wq_, bq_, Wk_, bk_ = (np.asarray(a, dtype=f) for a in (Wq, bq, Wk, bk))
    maps = []
    for b in range(B):
        m = dict(shared)
        m["xkt"] = np.asarray(fns["xkt"](ini_k[b].reshape(ST, H)))
        m["qkt"] = np.asarray(fns["qkt"](ini_q[b], Wq_, bq_, Wk_))
        m["bkq"] = np.asarray(fns["bkq"](ini_q[b], Wq_, bq_, bk_))
        m["mnegt"] = np.ascontiguousarray(mask[b].T * f(-10000.0))
        maps.append(m)
    return maps


def _seg_consts():
    import ml_dtypes
    bfdt = ml_dtypes.bfloat16
    if V3_BCAST == "ps":
        seg = np.zeros((128, 3 * 97), np.float32)
        for b in range(3):
            seg[0:TP, 97 * b + 32 * b] = 1.0
            seg[TP:128, 97 * b + 96] = 1.0
    else:
        seg = np.zeros((128, 12), np.float32)
        for b in range(3):
            seg[0:TP, 4 * b + b] = 1.0
            seg[TP:128, 4 * b + 3] = 1.0
    segt = np.zeros((4, 384), np.float32)
    for b in range(3):
        segt[b, 128 * b:128 * b + TP] = 1.0
        segt[3, 128 * b + TP:128 * (b + 1)] = 1.0
    return seg.astype(bfdt), segt.astype(bfdt)


def _in_maps3(ini_q, ini_k, mask, Wq, bq, Wk, bk, Wv, bv, Wo, bo, gamma,
              beta):
    import ml_dtypes
    f = np.float32
    bfdt = ml_dtypes.bfloat16
    fns = _prep_fns()

    wvt = np.asarray(Wv, dtype=f).T.astype(bfdt)
    wot = np.asarray(Wo, dtype=f).T.astype(bfdt)
    seg, segt = _seg_consts()
    shared = dict(
        wvt=np.ascontiguousarray(wvt),
        wot=np.ascontiguousarray(wot),
        seg3=seg, segt3=segt,
        bvr=np.asarray(bv, dtype=f).reshape(1, H).astype(bfdt),
        bor=np.asarray(bo, dtype=f).reshape(1, H).astype(bfdt),
        gam=np.asarray(gamma, dtype=f).reshape(1, H),
        bet=np.asarray(beta, dtype=f).reshape(1, H),
    )
    ini_q = np.asarray(ini_q, dtype=f)
    ini_k = np.asarray(ini_k, dtype=f)
    mask = np.asarray(mask, dtype=f)
    Wq_, bq_, Wk_, bk_ = (np.asarray(a, dtype=f) for a in (Wq, bq, Wk, bk))
    maps = []
    for b in range(B):
        m = dict(shared)
        m["xkt3"] = np.asarray(fns["xkt3"](ini_k[b], mask[b]))
        m["mneg3"] = np.asarray(fns["mneg3"](mask[b]))
        m["qkt"] = np.asarray(fns["qkt"](ini_q[b], Wq_, bq_, Wk_))
        m["bkq"] = np.asarray(fns["bkq"](ini_q[b], Wq_, bq_, bk_))
        maps.append(m)
    return maps


def run(inputs, loop=1, full_results=False, stages=9, ver=None):
    """Run the SPMD kernel; returns (B, Q, S, H) float32."""
    from concourse.bass_utils import run_bass_kernel_spmd

    if ver is None:
        cnt_max = int((np.asarray(inputs["mask"]) == 0).sum(-1).max())
        ver = 3 if cnt_max <= TP else 2
    flags = dict(
        stages=stages,
        bias_kq=bool(np.any(inputs["bq"]) or np.any(inputs["bk"])),
        bias_v=bool(np.any(inputs["bv"])),
        bias_o=bool(np.any(inputs["bo"])),
        gamma_beta=bool(np.any(np.asarray(inputs["gamma"]) != 1.0)
                        or np.any(inputs["beta"])),
    )
    nc = _get(loop=loop, ver=ver, **flags)
    maps = _in_maps3(**inputs) if ver == 3 else _in_maps(**inputs)
    err = None
    for _ in range(4):
        try:
            res = run_bass_kernel_spmd(nc, maps, list(range(NCORES)))
        except Exception as e:  # transient NRT device errors: retry
            err = e
            import time as _t
            _t.sleep(2.0)
            continue
        if full_results:
            return res
        # device output is s-major [S, Q, H]; transpose back to [Q, S, H]
        out = np.stack([res.results[c]["out"].transpose(1, 0, 2)
                        for c in range(NCORES)], axis=0)
        # transient first-execution corruption has been observed once on
        # this fleet; non-finite output -> re-execute
        if np.isfinite(out).all():
            return out
        err = RuntimeError("non-finite kernel output")
    raise err


def kernel(**inputs):
    return run(inputs, loop=1)

